# revision 2
# baseline (speedup 1.0000x reference)
"""Trainium2 Bass kernel for nn_HarmonicEstimation (topk_masking).

Problem: x [16,1,1025,1024] f32 -> mask [16,1,1025,1024].
Per (batch, t) column over f-bins 1..1024: find top-5 peaks, f0 = min index
among peaks with value > 0.1 (else 0); output column = harmonic-comb mask
that depends ONLY on f0.

Strategy (8 cores, 2 batches/core, no communication):
  - The output column is a pure function of f0, so precompute on host a
    transposed LUT  LUTT[k, f0]  (1025x1025 padded to 1040 rows, f32) and
    ship it to SBUF once per core (~4.3 MB).
  - Load input tiles in natural [f-part, t-free] layout (contiguous DMA),
    transpose to [t-part, f-free] via PE identity matmuls (PSUM), evacuate
    with the scalar engine.
  - DVE InstMax/InstMaxIndex give per-column top-8 values + indices in one
    pass each; tiny ops derive exact f0 (ties resolved like jax.lax.top_k).
    Columns with no peak > 0.1 (or f0 > 1020) map to LUT column with all
    0.5 entries via the f0=1024 sentinel.
  - GPSIMD ap_gather selects LUTT[:, f0[t]] per k-tile, directly emitting
    natural-layout [f-part, t-free] output tiles; DMA them out contiguous.
"""

import os
import sys

for _p in ("/opt/trn_rl_repo", "/root/.axon_site/_ro/trn_rl_repo"):
    if os.path.isdir(_p) and _p not in sys.path:
        sys.path.insert(0, _p)

import numpy as np

import concourse.bacc as bacc
import concourse.mybir as mybir
from concourse.bass_utils import run_bass_kernel_spmd
from concourse.tile import TileContext

dt = mybir.dt
Alu = mybir.AluOpType

B = 16          # full batch
NB = 2          # batches per core
NCORES = 8
F = 1025        # freq bins (0..1024)
T = 1024        # time columns
FT = 8          # f tiles of 128 covering bins 1..1024
TT = 8          # t tiles of 128
KTILES = 9      # output k tiles: 8 full + 1 row (k=1024)
LUT_ROWS = 1040  # 1025 k-rows padded to 16 partitions for the last gather
MAX_POWER = 0.1

_CACHE = {}


def _build_lutt() -> np.ndarray:
    """LUTT[k, f0] = reference mask value at bin k given fundamental f0.

    Mirrors reference.py arithmetic in float32 exactly. f0=0 and f0>1020
    both yield the all-0.5 column (ok=False everywhere).
    """
    if "lutt" in _CACHE:
        return _CACHE["lutt"]
    k = np.arange(F, dtype=np.int64)[:, None]       # [k, 1]
    f0 = np.arange(F, dtype=np.int64)[None, :]      # [1, f0]
    f0safe = np.maximum(f0, 1)
    limit = F - 3 - 2  # F - FREQ_MARGIN - 2 = 1020
    m_mult = np.minimum((k + 3) // f0safe, limit // f0safe)
    i_last = m_mult * f0safe
    dist = np.abs(k - i_last).astype(np.float32)
    val = np.maximum(
        np.float32(1.0) - (np.float32(0.5) * dist) / np.float32(3.0),
        np.float32(0.5),
    )
    ok = (f0 > 0) & (i_last >= f0safe) & (i_last >= k - 3)
    lutt = np.where(ok, val, np.float32(0.5)).astype(np.float32)  # [k, f0]
    out = np.full((LUT_ROWS, F), 0.5, dtype=np.float32)
    out[:F] = lutt
    _CACHE["lutt"] = out
    return out


def _build_nc():
    if "nc" in _CACHE:
        return _CACHE["nc"]
    from contextlib import ExitStack

    nc = bacc.Bacc("TRN2", target_bir_lowering=False, debug=False)
    x_in = nc.dram_tensor("x", [NB, F, T], dt.float32, kind="ExternalInput").ap()
    lutt_d = nc.dram_tensor("lutt", [LUT_ROWS, F], dt.float32, kind="ExternalInput").ap()
    ident_d = nc.dram_tensor("ident", [128, 128], dt.float32, kind="ExternalInput").ap()
    out_d = nc.dram_tensor("out", [NB, F, T], dt.float32, kind="ExternalOutput").ap()

    with TileContext(nc) as tc, ExitStack() as ctx:
        lutt_pool = ctx.enter_context(tc.tile_pool(name="luttp", bufs=1))
        const_pool = ctx.enter_context(tc.tile_pool(name="constp", bufs=1))
        nat_pool = ctx.enter_context(tc.tile_pool(name="natp", bufs=9))
        xt_pool = ctx.enter_context(tc.tile_pool(name="xtp", bufs=10))
        psum_pool = ctx.enter_context(tc.tile_pool(name="psump", bufs=6, space="PSUM"))
        small_pool = ctx.enter_context(tc.tile_pool(name="smallp", bufs=2))
        gath_pool = ctx.enter_context(tc.tile_pool(name="gathp", bufs=4))

        ident_sb = const_pool.tile([128, 128], dt.float32, name="ident_sb")
        nc.sync.dma_start(ident_sb[:], ident_d[:])

        lutt_sb = lutt_pool.tile([128, KTILES * F], dt.float32, name="lutt_sb")
        for h in range(8):
            nc.sync.dma_start(
                lutt_sb[:, h * F:(h + 1) * F], lutt_d[h * 128:(h + 1) * 128, :]
            )
        nc.sync.dma_start(lutt_sb[0:16, 8 * F:9 * F], lutt_d[1024:LUT_ROWS, :])

        for b in range(NB):
            # ---- load + transpose to [t-part, f-free] ----
            xts = [
                xt_pool.tile([128, FT * 128], dt.float32, name=f"xt{b}_{g}", tag="xt")
                for g in range(TT)
            ]
            for ftg in range(2):  # two groups of 4 f-tiles
                nats = []
                for j in range(4):
                    ft = ftg * 4 + j
                    nat = nat_pool.tile([128, T], dt.float32, name=f"nat{b}_{ft}", tag="nat")
                    nc.sync.dma_start(
                        nat[:], x_in[b, 1 + ft * 128: 1 + (ft + 1) * 128, :]
                    )
                    nats.append(nat)
                for g in range(TT):
                    ps = psum_pool.tile(
                        [128, 512], dt.float32, name=f"ps{b}_{ftg}_{g}", tag="ps"
                    )
                    for j in range(4):
                        nc.tensor.transpose(
                            ps[:, j * 128:(j + 1) * 128],
                            nats[j][:, g * 128:(g + 1) * 128],
                            ident_sb[:],
                        )
                    nc.scalar.copy(xts[g][:, ftg * 512:(ftg + 1) * 512], ps[:])

            # ---- top-8 per column ----
            vals = small_pool.tile([128, 64], dt.float32, name=f"vals{b}", tag="vals")
            idxs = small_pool.tile([128, 64], dt.uint32, name=f"idxs{b}", tag="idxs")
            for g in range(TT):
                nc.vector.max(vals[:, 8 * g:8 * g + 8], xts[g][:])
                nc.vector.max_index(idxs[:, 8 * g:8 * g + 8], vals[:, 8 * g:8 * g + 8], xts[g][:])

            # ---- exact f0 (slot 0..4 = top-5; +1 bin offset; >0.1 gate) ----
            idxp = small_pool.tile([128, 64], dt.uint32, name=f"idxp{b}", tag="idxp")
            mask = small_pool.tile([128, 64], dt.uint32, name=f"mask{b}", tag="mask")
            cand = small_pool.tile([128, 64], dt.uint32, name=f"cand{b}", tag="cand")
            f0u = small_pool.tile([128, 8], dt.uint32, name=f"f0u{b}", tag="f0u")
            f0h = small_pool.tile([128, 8], dt.int16, name=f"f0h{b}", tag="f0h")
            nc.vector.tensor_scalar(idxp[:], idxs[:], 1, None, Alu.add)
            nc.vector.tensor_scalar(mask[:], vals[:], float(MAX_POWER), None, Alu.is_gt)
            nc.vector.memset(cand[:], 1024)
            nc.vector.copy_predicated(cand[:], mask[:], idxp[:])
            cand_v = cand[:].rearrange("p (g s) -> p g s", s=8)[:, :, 0:5]
            nc.vector.tensor_reduce(
                f0u[:], cand_v, axis=mybir.AxisListType.X, op=Alu.min
            )
            nc.vector.tensor_copy(f0h[:], f0u[:])

            # ---- wrapped int16 index list for ap_gather ----
            # wrapped[q, g*8+a] = f0(t = g*128 + a*16 + q) ; replicate to all
            # 16-partition groups (each GPSIMD core reads its own 16 rows).
            wrapped = small_pool.tile([128, 64], dt.int16, name=f"wrap{b}", tag="wrap")
            wv = wrapped[:].rearrange("p (g a) -> p g a", a=8)
            with nc.allow_non_contiguous_dma("tiny f0 index shuffle"):
                for a in range(8):
                    nc.scalar.dma_start(wv[0:16, :, a:a + 1], f0h[16 * a:16 * (a + 1), :])
            nc.scalar.dma_start(wrapped[16:32, :], wrapped[0:16, :])
            nc.scalar.dma_start(wrapped[32:64, :], wrapped[0:32, :])
            nc.scalar.dma_start(wrapped[64:128, :], wrapped[0:64, :])

            # ---- gather LUTT columns -> natural output tiles -> DMA out ----
            for h in range(KTILES):
                chans = 128 if h < 8 else 16
                rows = 128 if h < 8 else 1
                gt = gath_pool.tile([128, T], dt.float32, name=f"gt{b}_{h}", tag="gath")
                nc.gpsimd.ap_gather(
                    gt[0:chans, :],
                    lutt_sb[0:chans, h * F:(h + 1) * F],
                    wrapped[0:chans, :],
                    channels=chans,
                    num_elems=F,
                    d=1,
                    num_idxs=T,
                )
                nc.sync.dma_start(
                    out_d[b, h * 128:h * 128 + rows, :], gt[0:rows, :]
                )

    nc.compile()
    _CACHE["nc"] = nc
    return nc


def kernel(x: np.ndarray) -> np.ndarray:
    x = np.asarray(x)
    assert x.shape == (B, 1, F, T), x.shape
    nc = _build_nc()
    lutt = _build_lutt()
    ident = np.eye(128, dtype=np.float32)
    in_maps = [
        {
            "x": np.ascontiguousarray(x[NB * c:NB * (c + 1), 0]),
            "lutt": lutt,
            "ident": ident,
        }
        for c in range(NCORES)
    ]
    res = run_bass_kernel_spmd(nc, in_maps, core_ids=list(range(NCORES)))
    out = np.concatenate([res.results[c]["out"] for c in range(NCORES)], axis=0)
    return out[:, None, :, :].astype(np.float32, copy=False)


# revision 3
# speedup vs baseline: 3.7347x; 3.7347x over previous
"""Trainium2 Bass kernel for nn_HarmonicEstimation (topk_masking).

Problem: x [16,1,1025,1024] f32 -> mask [16,1,1025,1024].
Per (batch, t) column over f-bins 1..1024: find top-5 peaks, f0 = min index
among peaks with value > 0.1 (else 0); output column = harmonic-comb mask
that depends ONLY on f0.

Strategy (8 cores, 2 batches/core, no communication):
  - The output column is a pure function of f0, so precompute on host a
    LUT[f0, k] (1025 x 1088-padded f32 rows) kept in DRAM.
  - Load input tiles in natural [f-part, t-free] layout (contiguous DMA),
    transpose to [t-part, f-free] via PE identity matmuls (PSUM), evacuate
    with the scalar engine.
  - DVE InstMax/InstMaxIndex give per-column top-8 values + indices in one
    pass each; tiny ops derive exact f0 (ties resolved like jax.lax.top_k).
    Columns with no peak > 0.1 (or f0 > 1020) map to the all-0.5 LUT row
    via the f0=1024 sentinel.
  - gpsimd dma_gather pulls LUT rows (one 4.25KB row per column) from DRAM
    into [t-part, k-free] SBUF tiles via the DMA engines; PE transposes
    them back to natural [k-part, t-free] tiles which DMA out contiguous.
    (GPSIMD ap_gather was tried first: ~28us per tile on HW - Q7 cores do
    16 partitions serially - so the gather must ride the DMA engines.)
"""

import os
import sys

for _p in ("/opt/trn_rl_repo", "/root/.axon_site/_ro/trn_rl_repo"):
    if os.path.isdir(_p) and _p not in sys.path:
        sys.path.insert(0, _p)

import numpy as np

import concourse.bacc as bacc
import concourse.mybir as mybir
from concourse.bass_utils import run_bass_kernel_spmd
from concourse.tile import TileContext

dt = mybir.dt
Alu = mybir.AluOpType

B = 16          # full batch
NB = 2          # batches per core
NCORES = 8
F = 1025        # freq bins (0..1024)
T = 1024        # time columns
FT = 8          # f tiles of 128 covering bins 1..1024
TT = 8          # t tiles of 128
LUT_W = 1088    # LUT row padded to 1088 f32 = 4352 B (multiple of 256)
NQ = 4          # SWDGE queues (ucode max)
MAX_POWER = 0.1

_CACHE = {}


def _build_lut() -> np.ndarray:
    """LUT[f0, k] = reference mask value at bin k given fundamental f0.

    Mirrors reference.py arithmetic in float32 exactly. f0=0 and f0>1020
    both yield the all-0.5 row (ok=False everywhere).
    """
    if "lut" in _CACHE:
        return _CACHE["lut"]
    k = np.arange(F, dtype=np.int64)[None, :]       # [1, k]
    f0 = np.arange(F, dtype=np.int64)[:, None]      # [f0, 1]
    f0safe = np.maximum(f0, 1)
    limit = F - 3 - 2  # F - FREQ_MARGIN - 2 = 1020
    m_mult = np.minimum((k + 3) // f0safe, limit // f0safe)
    i_last = m_mult * f0safe
    dist = np.abs(k - i_last).astype(np.float32)
    val = np.maximum(
        np.float32(1.0) - (np.float32(0.5) * dist) / np.float32(3.0),
        np.float32(0.5),
    )
    ok = (f0 > 0) & (i_last >= f0safe) & (i_last >= k - 3)
    lut = np.where(ok, val, np.float32(0.5)).astype(np.float32)  # [f0, k]
    out = np.full((F, LUT_W), 0.5, dtype=np.float32)
    out[:, :F] = lut
    _CACHE["lut"] = out
    return out


def _build_nc():
    if "nc" in _CACHE:
        return _CACHE["nc"]
    from contextlib import ExitStack

    nc = bacc.Bacc(
        "TRN2", target_bir_lowering=False, debug=False, num_swdge_queues=NQ
    )
    x_in = nc.dram_tensor("x", [NB, F, T], dt.float32, kind="ExternalInput").ap()
    lut_d = nc.dram_tensor("lut", [F, LUT_W], dt.float32, kind="ExternalInput").ap()
    ident_d = nc.dram_tensor("ident", [128, 128], dt.float32, kind="ExternalInput").ap()
    out_d = nc.dram_tensor("out", [NB, F, T], dt.float32, kind="ExternalOutput").ap()

    with TileContext(nc) as tc, ExitStack() as ctx:
        const_pool = ctx.enter_context(tc.tile_pool(name="constp", bufs=1))
        nat_pool = ctx.enter_context(tc.tile_pool(name="natp", bufs=9))
        xt_pool = ctx.enter_context(tc.tile_pool(name="xtp", bufs=10))
        gg_pool = ctx.enter_context(tc.tile_pool(name="ggp", bufs=10))
        gout_pool = ctx.enter_context(tc.tile_pool(name="goutp", bufs=4))
        psum_pool = ctx.enter_context(tc.tile_pool(name="psump", bufs=3, space="PSUM"))
        small_pool = ctx.enter_context(tc.tile_pool(name="smallp", bufs=2))

        ident_sb = const_pool.tile([128, 128], dt.float32, name="ident_sb")
        nc.sync.dma_start(ident_sb[:], ident_d[:])

        for b in range(NB):
            # ---- load + transpose to [t-part, f-free] ----
            xts = [
                xt_pool.tile([128, FT * 128], dt.float32, name=f"xt{b}_{g}", tag="xt")
                for g in range(TT)
            ]
            for ftg in range(2):  # two groups of 4 f-tiles
                nats = []
                for j in range(4):
                    ft = ftg * 4 + j
                    nat = nat_pool.tile([128, T], dt.float32, name=f"nat{b}_{ft}", tag="nat")
                    nc.sync.dma_start(
                        nat[:], x_in[b, 1 + ft * 128: 1 + (ft + 1) * 128, :]
                    )
                    nats.append(nat)
                for g in range(TT):
                    ps = psum_pool.tile(
                        [128, 512], dt.float32, name=f"ps{b}_{ftg}_{g}", tag="ps"
                    )
                    for j in range(4):
                        nc.tensor.transpose(
                            ps[:, j * 128:(j + 1) * 128],
                            nats[j][:, g * 128:(g + 1) * 128],
                            ident_sb[:],
                        )
                    nc.scalar.copy(xts[g][:, ftg * 512:(ftg + 1) * 512], ps[:])

            # ---- top-8 per column ----
            vals = small_pool.tile([128, 64], dt.float32, name=f"vals{b}", tag="vals")
            idxs = small_pool.tile([128, 64], dt.uint32, name=f"idxs{b}", tag="idxs")
            for g in range(TT):
                nc.vector.max(vals[:, 8 * g:8 * g + 8], xts[g][:])
                nc.vector.max_index(idxs[:, 8 * g:8 * g + 8], vals[:, 8 * g:8 * g + 8], xts[g][:])

            # ---- exact f0 (slot 0..4 = top-5; +1 bin offset; >0.1 gate) ----
            idxp = small_pool.tile([128, 64], dt.uint32, name=f"idxp{b}", tag="idxp")
            mask = small_pool.tile([128, 64], dt.uint32, name=f"mask{b}", tag="mask")
            cand = small_pool.tile([128, 64], dt.uint32, name=f"cand{b}", tag="cand")
            f0u = small_pool.tile([128, 8], dt.uint32, name=f"f0u{b}", tag="f0u")
            f0h = small_pool.tile([128, 8], dt.int16, name=f"f0h{b}", tag="f0h")
            nc.vector.tensor_scalar(idxp[:], idxs[:], 1, None, Alu.add)
            nc.vector.tensor_scalar(mask[:], vals[:], float(MAX_POWER), None, Alu.is_gt)
            nc.vector.memset(cand[:], 1024)
            nc.vector.copy_predicated(cand[:], mask[:], idxp[:])
            cand_v = cand[:].rearrange("p (g s) -> p g s", s=8)[:, :, 0:5]
            nc.vector.tensor_reduce(
                f0u[:], cand_v, axis=mybir.AxisListType.X, op=Alu.min
            )
            nc.vector.tensor_copy(f0h[:], f0u[:])

            # ---- wrapped int16 index list for dma_gather ----
            # wrapped[q, g*8+a] = f0(t = g*128 + a*16 + q) ; replicate to all
            # 16-partition groups (descriptor gen reads per-16-row groups).
            wrapped = small_pool.tile([128, 64], dt.int16, name=f"wrap{b}", tag="wrap")
            wv = wrapped[:].rearrange("p (g a) -> p g a", a=8)
            with nc.allow_non_contiguous_dma("tiny f0 index shuffle"):
                for a in range(8):
                    nc.scalar.dma_start(wv[0:16, :, a:a + 1], f0h[16 * a:16 * (a + 1), :])
            nc.scalar.dma_start(wrapped[16:32, :], wrapped[0:16, :])
            nc.scalar.dma_start(wrapped[32:64, :], wrapped[0:32, :])
            nc.scalar.dma_start(wrapped[64:128, :], wrapped[0:64, :])

            # ---- gather LUT rows (DMA engines) -> [t-part, k-free] tiles ----
            ggs = []
            for g in range(TT):
                gg = gg_pool.tile([128, LUT_W], dt.float32, name=f"gg{b}_{g}", tag="gg")
                nc.gpsimd.dma_gather(
                    gg[:].rearrange("p (c e) -> p c e", c=1),
                    lut_d[:],
                    wrapped[:, 8 * g:8 * g + 8],
                    num_idxs=128,
                    num_idxs_reg=128,
                    elem_size=LUT_W,
                    queue_num=(b * TT + g) % NQ,
                )
                ggs.append(gg)

            # ---- transpose back to natural [k-part, t-free] + write out ----
            for h in range(FT):
                out_nat = gout_pool.tile([128, T], dt.float32, name=f"on{b}_{h}", tag="onat")
                for half in range(2):
                    pso = psum_pool.tile(
                        [128, 512], dt.float32, name=f"pso{b}_{h}_{half}", tag="pso"
                    )
                    for j in range(4):
                        g = half * 4 + j
                        nc.tensor.transpose(
                            pso[:, j * 128:(j + 1) * 128],
                            ggs[g][:, h * 128:(h + 1) * 128],
                            ident_sb[:],
                        )
                    nc.scalar.copy(out_nat[:, half * 512:(half + 1) * 512], pso[:])
                nc.sync.dma_start(out_d[b, h * 128:(h + 1) * 128, :], out_nat[:])
            # k=1024 row: strided tiny DMAs straight from the gathered tiles
            with nc.allow_non_contiguous_dma("last output row"):
                for g in range(TT):
                    nc.sync.dma_start(
                        out_d[b, 1024:1025, g * 128:(g + 1) * 128],
                        ggs[g][:, 1024:1025],
                    )

    nc.compile()
    _CACHE["nc"] = nc
    return nc


def kernel(x: np.ndarray) -> np.ndarray:
    x = np.asarray(x)
    assert x.shape == (B, 1, F, T), x.shape
    nc = _build_nc()
    lut = _build_lut()
    ident = np.eye(128, dtype=np.float32)
    in_maps = [
        {
            "x": np.ascontiguousarray(x[NB * c:NB * (c + 1), 0]),
            "lut": lut,
            "ident": ident,
        }
        for c in range(NCORES)
    ]
    res = run_bass_kernel_spmd(nc, in_maps, core_ids=list(range(NCORES)))
    out = np.concatenate([res.results[c]["out"] for c in range(NCORES)], axis=0)
    return out[:, None, :, :].astype(np.float32, copy=False)
